# revision 44
# baseline (speedup 1.0000x reference)
"""Stein solver  Lambda - A @ Lambda @ W = C @ Y  on 8 trn2 NeuronCores.

Math: Lambda = sum_k A^k R W^k with R = C@Y; per-step Frobenius contraction
of the series terms is ~0.08, so a 3-term truncation has exact error 5.1e-4
(gate is 2e-2).  Computed as

    S = R + (U0 + T2) @ W,   U0 = (A@C)@Y = A R,   T2 = ((A@A@C)@Y)@W = A^2 R W

which needs NO inter-core collectives: every GEMM is either full x full or
own-rows x full.

Distribution: row-sharded over 8 cores, core c owns rows [128c, 128c+128).
Five passes (496 matmuls + 80 transposes per core):
  P1: A2 = A@A + V = A@C, both 4-mult fp8(e4m3), sharing one fp8 A stream
      and the fp8 C resident; one shared 3-plane stationary (r, i, -i).
  P2: R = C@Y (bf16) + U0 = V@Y (bf16) + V2 = A2@C (fp8 DoubleRow),
      all 4-mult, sharing ONE streamed bf16-Y pass (6 of 8 PSUM banks).
  P3: U2 = V2@Y (4-mult fp8 DoubleRow, fully prefetched fp8 Y).
  P4: T2 = U2@W (Karatsuba bf16, W resident); combine writes M = U0 + T2.
  P5: S = R + M@W (Karatsuba bf16); combine adds R and DMAs the output.

fp8 keeps every pass under ~200 GB/s of live DMA (the bf16 version was
HBM-starved in P1/P3); operands carry host-folded power-of-2 scales
(A x32, C x16, Y x16, A2T x128, V2T x64) that are divided back out in the
PSUM-drain copies (scalar.mul is a free scaled copy).
fp8 terms enter the answer at <= 8e-2 relative scale; measured end-to-end
error of this exact scheme (CPU simulation): 3.9e-3 vs the 2e-2 gate.

DMA engine split: sync(SP)-DGE carries the latency-critical streams (A
tiles, Y tiles, output), scalar(Act)-DGE the C resident + CTw, and
gpsimd SWDGE the 6MB W resident, gated on P1's first combine so it cannot
steal HBM bandwidth from the P1 streams.  PSUM-sourced vector ops always
have at most one PSUM operand; drains alternate scalar/vector engines.
"""

import numpy as np

P = 128
N = 1024
KT = N // P          # 8 k-tiles
NC = 8               # cores
NCH = 2              # 512-wide n-chunks per 1024-col output row block
CW = N // NCH        # 512

SA = 32.0            # fp8 scale on A planes
SC = 16.0            # fp8 scale on C planes
SY = 16.0            # fp8 scale on Y planes
SA2 = 128.0          # fp8 scale on A2T planes
SV2 = 64.0           # fp8 scale on V2T planes

_compiled = {}


def _build():
    import concourse.mybir as mybir
    import concourse.tile as tile
    from concourse import bacc
    from concourse.masks import make_identity

    f32 = mybir.dt.float32
    f32r = mybir.dt.float32r
    bf16 = mybir.dt.bfloat16
    f8 = mybir.dt.float8e4
    DR = mybir.MatmulPerfMode.DoubleRow

    nc = bacc.Bacc("TRN2", target_bir_lowering=False, debug=False, num_devices=NC)

    # ---- I/O ----
    # full moving matrices laid out [partition, plane, ktile, col]:
    #   X[kt*128+p, c] at [p, j, kt, c]
    # sharded stationary [partition, plane, ktile, m]: (X[own,:].T) blocks
    ATq = nc.dram_tensor("ATq", [P, 3, KT, P], f8, kind="ExternalInput")       # x32: r,i,-i
    CTq = nc.dram_tensor("CTq", [P, 3, KT, P], bf16, kind="ExternalInput")     # r,i,-i
    Af = nc.dram_tensor("Af", [P, 2, KT, N], f8, kind="ExternalInput")         # x32: r,i
    Cf = nc.dram_tensor("Cf", [P, 2, KT, N], f8, kind="ExternalInput")         # x16: r,i
    Yfr = nc.dram_tensor("Yfr", [P, 2, KT, N], bf16, kind="ExternalInput")     # r,i
    Yfb = nc.dram_tensor("Yfb", [P, 2, KT, N], f8, kind="ExternalInput")       # x16: r,i
    Wf = nc.dram_tensor("Wf", [P, 3, KT, N], bf16, kind="ExternalInput")       # r,i,r+i
    out = nc.dram_tensor("out", [2, P, N], f32, kind="ExternalOutput")

    with tile.TileContext(nc) as tc:
        with (
            tc.tile_pool(name="res", bufs=1) as res,          # residents + stationaries
            tc.tile_pool(name="stat", bufs=2) as statp,       # rotating transposed weights
            tc.tile_pool(name="work", bufs=2) as workp,       # rotating bf16 work tiles
            tc.tile_pool(name="mov", bufs=3) as movp,         # streamed moving tiles
            tc.tile_pool(name="tmp", bufs=3) as tmpp,         # combine temporaries
            tc.tile_pool(name="psum", bufs=6, space="PSUM") as ppool,
            tc.tile_pool(name="tpsum", bufs=2, space="PSUM") as tppool,
        ):
            identf = res.tile([P, P], f32, tag="identf")
            make_identity(nc, identf)
            identb = res.tile([P, P], bf16, tag="identb")
            nc.vector.tensor_copy(identb[:], identf[:])

            # PE warmup: keep the tensor engine busy (p-state ramp) while the
            # first DMAs land.  Results are discarded.
            for _ in range(10):
                wtp = tppool.tile([P, P], bf16, tag="tpb", name="wtp")
                nc.tensor.transpose(wtp[:], identb[:], identb[:])

            # sync(SP)-DGE: only the P1-critical small loads + the A stream
            ATw = res.tile([P, 3, KT, P], f8, tag="ATw")
            nc.sync.dma_start(ATw[:], ATq.ap())
            # scalar(Act)-DGE: C resident as ONE transfer (P1 runs all A2
            # matmuls before any V matmul, so Cres has ~13us to land and
            # chunk-granularity only added queue round-trips), then CTw.
            Cres = res.tile([P, 2, KT, N], f8, tag="Cres")
            nc.scalar.dma_start(Cres[:], Cf.ap())
            CTw = res.tile([P, 3, KT, P], bf16, tag="CTw")
            nc.scalar.dma_start(CTw[:], CTq.ap())

            afa = Af.ap()
            yra = Yfr.ap()
            yba = Yfb.ap()
            wfa = Wf.ap()

            def kara_combine(pk, cb):
                """pk = [P1, P2, P3] psums; cb(re_fn, im_fn) where the fns
                write re = P1-P2, im = P3-P1-P2 (<=1 psum operand per op)."""
                t1 = tmpp.tile([P, CW], f32, tag="kt", name="t1")
                nc.scalar.copy(t1[:], pk[0][:])
                t3 = tmpp.tile([P, CW], f32, tag="kt", name="t3")
                nc.scalar.copy(t3[:], pk[2][:])
                u = tmpp.tile([P, CW], f32, tag="kt", name="u")
                nc.vector.tensor_sub(u[:], t3[:], t1[:])
                cb(lambda dst: nc.vector.tensor_sub(dst, t1[:], pk[1][:]),
                   lambda dst: nc.vector.tensor_sub(dst, u[:], pk[1][:]))

            def mm4(pb, wt, mov0, mov1, st, sp, pm=None):
                """4-mult complex accumulate: wt planes (r, i, -i)."""
                nc.tensor.matmul(pb[0][:], wt[0], mov0, start=st, stop=False, perf_mode=pm)
                nc.tensor.matmul(pb[0][:], wt[2], mov1, start=False, stop=sp, perf_mode=pm)
                nc.tensor.matmul(pb[1][:], wt[0], mov1, start=st, stop=False, perf_mode=pm)
                nc.tensor.matmul(pb[1][:], wt[1], mov0, start=False, stop=sp, perf_mode=pm)

            # ---------------- P1: A2 = A@A + V = A@C (4m fp8) -------------
            A2b = workp.tile([P, 2, N], bf16, tag="wb", name="A2b")
            Vb = workp.tile([P, 2, N], bf16, tag="wb", name="Vb")
            for ci in range(NCH):
                cs = slice(CW * ci, CW * ci + CW)
                pa = [ppool.tile([P, CW], f32, tag="ps", name="pa") for _ in range(2)]
                pv = [ppool.tile([P, CW], f32, tag="ps", name="pv") for _ in range(2)]
                # all A2 matmuls (DMA-fed) before any V matmul (SBUF-fed):
                # an in-order PE queue must never park a V matmul that waits
                # on Cres in front of ready A2 work.
                for t in range(KT):
                    at = movp.tile([P, 2, CW], f8, tag="at", name="at")
                    nc.sync.dma_start(at[:], afa[:, :, t, cs])
                    wt = [ATw[:, j, t] for j in range(3)]
                    mm4(pa, wt, at[:, 0, :], at[:, 1, :], t == 0, t == KT - 1)
                for t in range(KT):
                    wt = [ATw[:, j, t] for j in range(3)]
                    mm4(pv, wt, Cres[:, 0, t, cs], Cres[:, 1, t, cs], t == 0, t == KT - 1)
                nc.scalar.mul(A2b[:, 0, cs], pa[0][:], 1.0 / (SA * SA))
                nc.vector.tensor_scalar_mul(A2b[:, 1, cs], pa[1][:], 1.0 / (SA * SA))
                nc.scalar.mul(Vb[:, 0, cs], pv[0][:], 1.0 / (SA * SC))
                nc.vector.tensor_scalar_mul(Vb[:, 1, cs], pv[1][:], 1.0 / (SA * SC))

            # W resident on the scalar HW-DGE, data-gated on P1's first
            # combine: a dummy tile shares Wres's buffer (same tag, bufs=1)
            # and is read by an op that depends on A2b, so the buffer-reuse
            # WAR dependency keeps the 6MB load from competing with the P1
            # streams for HBM.  (Engine-queue emission order alone is NOT a
            # schedule constraint - the tile list-scheduler reorders by
            # dependency readiness.)
            wdummy = res.tile([P, 3, KT, N], bf16, tag="Wres", name="wdummy")
            nc.gpsimd.memset(wdummy[0:1, 0, :, 0], 0.0)
            wgate = tmpp.tile([1, KT], f32, tag="wg", name="wgate", bufs=1)
            nc.vector.tensor_add(wgate[:], wdummy[0:1, 0, :, 0], A2b[0:1, 0, 0:KT])
            Wres = res.tile([P, 3, KT, N], bf16, tag="Wres", name="Wres")
            for t in range(KT):
                nc.scalar.dma_start(Wres[:, :, t, :], wfa[:, :, t, :])

            # A2T (fp8, x128, scalar-engine copies) and VT (f32r, vector-
            # engine copies) built in one interleaved loop so PE transposes
            # never wait on a single drain engine.
            A2T = res.tile([P, 3, KT, P], f8, tag="A2T")
            VT = res.tile([P, 3, KT, P], bf16, tag="VT")
            for t in range(KT):
                blk = slice(P * t, P * t + P)
                tpr = tppool.tile([P, P], bf16, tag="tpb", name="tpr")
                nc.tensor.transpose(tpr[:], A2b[:, 0, blk], identb[:])
                nc.scalar.mul(A2T[:, 0, t], tpr[:], SA2)
                tpi = tppool.tile([P, P], bf16, tag="tpb", name="tpi")
                nc.tensor.transpose(tpi[:], A2b[:, 1, blk], identb[:])
                nc.vector.tensor_scalar_mul(A2T[:, 1, t], tpi[:], SA2)
                nc.scalar.mul(A2T[:, 2, t], tpi[:], -SA2)
                vpr = tppool.tile([P, P], bf16, tag="tpb", name="vpr")
                nc.tensor.transpose(vpr[:], Vb[:, 0, blk], identb[:])
                nc.vector.tensor_copy(VT[:, 0, t], vpr[:])
                vpi = tppool.tile([P, P], bf16, tag="tpb", name="vpi")
                nc.tensor.transpose(vpi[:], Vb[:, 1, blk], identb[:])
                nc.vector.tensor_copy(VT[:, 1, t], vpi[:])
                nc.vector.tensor_scalar_mul(VT[:, 2, t], vpi[:], -1.0)

            # ---------------- P2: R = C@Y + U0 = V@Y + V2 = A2@C (4m) -----
            R32 = res.tile([P, 2, N], f32, tag="R32")
            U032 = res.tile([P, 2, N], f32, tag="U032")
            V2b = workp.tile([P, 2, N], bf16, tag="wb", name="V2b")
            for ci in range(NCH):
                cs = slice(CW * ci, CW * ci + CW)
                pr = [ppool.tile([P, CW], f32, tag="ps", name="pr") for _ in range(2)]
                pu = [ppool.tile([P, CW], f32, tag="ps", name="pu") for _ in range(2)]
                pv2 = [ppool.tile([P, CW], f32, tag="ps", name="pv2") for _ in range(2)]
                for t in range(KT):
                    yt = movp.tile([P, 2, CW], bf16, tag="yt", name="yt", bufs=4)
                    nc.sync.dma_start(yt[:], yra[:, :, t, cs])
                    st = t == 0
                    sp = t == KT - 1
                    mm4(pr, [CTw[:, j, t] for j in range(3)], yt[:, 0, :], yt[:, 1, :], st, sp)
                    mm4(pu, [VT[:, j, t] for j in range(3)], yt[:, 0, :], yt[:, 1, :], st, sp)
                    if t % 2 == 0:
                        # V2 in fp8 DoubleRow: one matmul covers the k-tile
                        # pair [t, t+1]; stationary/moving pair views are
                        # plain slices of the existing [.., KT, ..] layouts.
                        tp = slice(t, t + 2)
                        mm4(pv2, [A2T[:, j, tp, :] for j in range(3)],
                            Cres[:, 0, tp, cs], Cres[:, 1, tp, cs],
                            st, t == KT - 2, pm=DR)
                # drain the 6 banks on two engines in parallel
                nc.scalar.copy(R32[:, 0, cs], pr[0][:])
                nc.vector.tensor_copy(R32[:, 1, cs], pr[1][:])
                nc.scalar.copy(U032[:, 0, cs], pu[0][:])
                nc.vector.tensor_copy(U032[:, 1, cs], pu[1][:])
                nc.scalar.mul(V2b[:, 0, cs], pv2[0][:], 1.0 / (SA2 * SC))
                nc.vector.tensor_scalar_mul(V2b[:, 1, cs], pv2[1][:], 1.0 / (SA2 * SC))

            # V2T: fp8 x64 planes (r, i, -i) for the fp8 U2 pass
            V2T = statp.tile([P, 3, KT, P], f8, tag="v2t", name="V2T", bufs=1)
            for t in range(KT):
                blk = slice(P * t, P * t + P)
                tpr = tppool.tile([P, P], bf16, tag="tpb", name="w2r")
                nc.tensor.transpose(tpr[:], V2b[:, 0, blk], identb[:])
                nc.scalar.mul(V2T[:, 0, t], tpr[:], SV2)
                tpi = tppool.tile([P, P], bf16, tag="tpb", name="w2i")
                nc.tensor.transpose(tpi[:], V2b[:, 1, blk], identb[:])
                nc.vector.tensor_scalar_mul(V2T[:, 1, t], tpi[:], SV2)
                nc.scalar.mul(V2T[:, 2, t], tpi[:], -SV2)

            # ---------------- P3: U2 = V2@Y (4m fp8, DoubleRow) -----------
            # ybt pair-tiles get one buffer each (bufs=8 = total loads), so
            # the whole 2MB fp8 Y stream prefetches during P2 and P3 runs
            # with zero live DMA.
            U2b = workp.tile([P, 2, N], bf16, tag="wb", name="U2b")
            for ci in range(NCH):
                cs = slice(CW * ci, CW * ci + CW)
                pk = [ppool.tile([P, CW], f32, tag="ps", name="pk") for _ in range(2)]
                for tp2 in range(KT // 2):
                    tp = slice(2 * tp2, 2 * tp2 + 2)
                    ybt = movp.tile([P, 2, 2, CW], f8, tag="ybt", name="ybt", bufs=8)
                    nc.sync.dma_start(ybt[:], yba[:, :, tp, cs])
                    st = tp2 == 0
                    sp = tp2 == KT // 2 - 1
                    mm4(pk, [V2T[:, j, tp, :] for j in range(3)],
                        ybt[:, 0, :, :], ybt[:, 1, :, :], st, sp, pm=DR)
                nc.scalar.mul(U2b[:, 0, cs], pk[0][:], 1.0 / (SV2 * SY))
                nc.vector.tensor_scalar_mul(U2b[:, 1, cs], pk[1][:], 1.0 / (SV2 * SY))

            def tr_kara_t(src_bf, wt, t):
                """One k-tile of bf16 [P,2,N] -> Karatsuba planes (r,i,r+i)."""
                blk = slice(P * t, P * t + P)
                tpr = tppool.tile([P, P], bf16, tag="tpb", name="kr")
                nc.tensor.transpose(tpr[:], src_bf[:, 0, blk], identb[:])
                nc.scalar.copy(wt[:, 0, t], tpr[:])
                tpi = tppool.tile([P, P], bf16, tag="tpb", name="ki")
                nc.tensor.transpose(tpi[:], src_bf[:, 1, blk], identb[:])
                nc.vector.tensor_copy(wt[:, 1, t], tpi[:])
                nc.vector.tensor_add(wt[:, 2, t], wt[:, 0, t], tpi[:])

            U2T = statp.tile([P, 3, KT, P], bf16, tag="wt", name="U2T")
            for t in range(KT):
                tr_kara_t(U2b, U2T, t)

            # ---------------- P4: T2 = U2@W (kara); M = U0 + T2 -----------
            Mb = workp.tile([P, 2, N], bf16, tag="wb", name="Mb")
            for ci in range(NCH):
                cs = slice(CW * ci, CW * ci + CW)
                pk = [ppool.tile([P, CW], f32, tag="ps", name="pt") for _ in range(3)]
                for t in range(KT):
                    st = t == 0
                    sp = t == KT - 1
                    nc.tensor.matmul(pk[0][:], U2T[:, 0, t], Wres[:, 0, t, cs], start=st, stop=sp)
                    nc.tensor.matmul(pk[1][:], U2T[:, 1, t], Wres[:, 1, t, cs], start=st, stop=sp)
                    nc.tensor.matmul(pk[2][:], U2T[:, 2, t], Wres[:, 2, t, cs], start=st, stop=sp)

                def cbm(re, im, cs=cs):
                    rr = tmpp.tile([P, CW], f32, tag="kt", name="rr")
                    re(rr[:])
                    nc.vector.tensor_add(Mb[:, 0, cs], rr[:], U032[:, 0, cs])
                    ii = tmpp.tile([P, CW], f32, tag="kt", name="ii")
                    im(ii[:])
                    nc.vector.tensor_add(Mb[:, 1, cs], ii[:], U032[:, 1, cs])

                kara_combine(pk, cbm)

            MT = statp.tile([P, 3, KT, P], bf16, tag="wt", name="MT")
            for t in range(KT):
                tr_kara_t(Mb, MT, t)

            # ---------------- P5: S = R + M@W; write out ------------------
            oa = out.ap()
            for ci in range(NCH):
                cs = slice(CW * ci, CW * ci + CW)
                pk = [ppool.tile([P, CW], f32, tag="ps", name="pf") for _ in range(3)]
                for t in range(KT):
                    st = t == 0
                    sp = t == KT - 1
                    nc.tensor.matmul(pk[0][:], MT[:, 0, t], Wres[:, 0, t, cs], start=st, stop=sp)
                    nc.tensor.matmul(pk[1][:], MT[:, 1, t], Wres[:, 1, t, cs], start=st, stop=sp)
                    nc.tensor.matmul(pk[2][:], MT[:, 2, t], Wres[:, 2, t, cs], start=st, stop=sp)

                def cbf(re, im, cs=cs):
                    for j, part in ((0, re), (1, im)):
                        pp = tmpp.tile([P, CW], f32, tag="kt", name="pp")
                        part(pp[:])
                        og = tmpp.tile([P, CW], f32, tag="og", name="og", bufs=2)
                        nc.vector.tensor_add(og[:], pp[:], R32[:, j, cs])
                        nc.sync.dma_start(oa[j, :, cs], og[:])

                kara_combine(pk, cbf)

    nc.compile()
    return nc


def _prep_inputs(A, W, C, Y):
    import ml_dtypes
    bf = ml_dtypes.bfloat16
    f8 = ml_dtypes.float8_e4m3fn

    def full_layout(planes, dt):
        pl = np.stack(planes)  # [p, 1024, 1024]
        return np.ascontiguousarray(
            pl.reshape(len(planes), KT, P, N).transpose(2, 0, 1, 3).astype(dt))

    def shard_layout(M, c, planes_fn, dt):
        XT = M[P * c:P * c + P, :].T
        r = XT.real.astype(np.float32)
        i = XT.imag.astype(np.float32)
        pl = np.stack(planes_fn(r, i))  # [p, 1024, 128]
        npl = pl.shape[0]
        return np.ascontiguousarray(
            pl.reshape(npl, KT, P, P).transpose(2, 0, 1, 3).astype(dt))

    def re_im(M):
        return M.real.astype(np.float32), M.imag.astype(np.float32)

    Ar, Ai = re_im(A)
    Cr, Ci = re_im(C)
    Yr, Yi = re_im(Y)
    Wr, Wi = re_im(W)

    Af = full_layout([SA * Ar, SA * Ai], f8)
    Cfull = full_layout([SC * Cr, SC * Ci], f8)
    Yfr = full_layout([Yr, Yi], bf)
    Yfb = full_layout([SY * Yr, SY * Yi], f8)
    Wfull = full_layout([Wr, Wi, Wr + Wi], bf)

    in_maps = []
    for c in range(NC):
        in_maps.append({
            "ATq": shard_layout(A, c, lambda r, i: [SA * r, SA * i, -SA * i], f8),
            "CTq": shard_layout(C, c, lambda r, i: [r, i, -i], bf),
            "Af": Af, "Cf": Cfull, "Yfr": Yfr, "Yfb": Yfb, "Wf": Wfull,
        })
    return in_maps


def kernel(A, W, C, Y, _trace=False):
    from concourse import bass_utils

    if "nc" not in _compiled:
        _compiled["nc"] = _build()
    nc = _compiled["nc"]

    in_maps = _prep_inputs(A, W, C, Y)
    res = bass_utils.run_bass_kernel_spmd(
        nc, in_maps, core_ids=list(range(NC)), trace=_trace
    )
    _compiled["last_result"] = res

    full = np.empty((N, N), dtype=np.complex128)
    for c in range(NC):
        o = res.results[c]["out"]
        full[P * c:P * c + P, :] = o[0].astype(np.float64) + 1j * o[1].astype(np.float64)
    return full


# revision 49
# speedup vs baseline: 1.1134x; 1.1134x over previous
"""Stein solver  Lambda - A @ Lambda @ W = C @ Y  on 8 trn2 NeuronCores.

Math: Lambda = sum_k A^k R W^k with R = C@Y; per-step Frobenius contraction
of the series terms is ~0.08, so a 3-term truncation has exact error 5.1e-4
(gate is 2e-2).  Computed as

    S = R + (U0 + T2) @ W,   U0 = (A@C)@Y = A R,   T2 = ((A@A@C)@Y)@W = A^2 R W

which needs NO inter-core collectives: every GEMM is either full x full or
own-rows x full.

Distribution: row-sharded over 8 cores, core c owns rows [128c, 128c+128).
Five passes (496 matmuls + 80 transposes per core):
  P1: A2 = A@A + V = A@C, both 4-mult fp8(e4m3), sharing one fp8 A stream
      and the fp8 C resident; one shared 3-plane stationary (r, i, -i).
  P2: R = C@Y (bf16) + U0 = V@Y (bf16) + V2 = A2@C (fp8 DoubleRow),
      all 4-mult, sharing ONE streamed bf16-Y pass (6 of 8 PSUM banks).
  P3: U2 = V2@Y (4-mult fp8 DoubleRow, fully prefetched fp8 Y).
  P4: T2 = U2@W (Karatsuba bf16, W resident); combine writes M = U0 + T2.
  P5: S = R + M@W (Karatsuba bf16); combine adds R and DMAs the output.

fp8 keeps every pass under ~200 GB/s of live DMA (the bf16 version was
HBM-starved in P1/P3); operands carry host-folded power-of-2 scales
(A x32, C x16, Y x16, A2T x128, V2T x64) that are divided back out in the
PSUM-drain copies (scalar.mul is a free scaled copy).
fp8 terms enter the answer at <= 8e-2 relative scale; measured end-to-end
error of this exact scheme (CPU simulation): 3.9e-3 vs the 2e-2 gate.

DMA engine split: sync(SP)-DGE carries the latency-critical streams (A
tiles, Y tiles, output), scalar(Act)-DGE the C resident + CTw, and
gpsimd SWDGE the 6MB W resident, gated on P1's first combine so it cannot
steal HBM bandwidth from the P1 streams.  PSUM-sourced vector ops always
have at most one PSUM operand; drains alternate scalar/vector engines.
"""

import numpy as np

P = 128
N = 1024
KT = N // P          # 8 k-tiles
NC = 8               # cores
NCH = 2              # 512-wide n-chunks per 1024-col output row block
CW = N // NCH        # 512

SA = 32.0            # fp8 scale on A planes
SC = 16.0            # fp8 scale on C planes
SY = 16.0            # fp8 scale on Y planes
SA2 = 128.0          # fp8 scale on A2T planes
SV2 = 64.0           # fp8 scale on V2T planes

_compiled = {}


def _build():
    import concourse.mybir as mybir
    import concourse.tile as tile
    from concourse import bacc
    from concourse.masks import make_identity

    f32 = mybir.dt.float32
    f32r = mybir.dt.float32r
    bf16 = mybir.dt.bfloat16
    f8 = mybir.dt.float8e4
    DR = mybir.MatmulPerfMode.DoubleRow

    nc = bacc.Bacc("TRN2", target_bir_lowering=False, debug=False, num_devices=NC)

    # ---- I/O ----
    # full moving matrices laid out [partition, plane, ktile, col]:
    #   X[kt*128+p, c] at [p, j, kt, c]
    # sharded stationary [partition, plane, ktile, m]: (X[own,:].T) blocks
    ATq = nc.dram_tensor("ATq", [P, 3, KT, P], f8, kind="ExternalInput")       # x32: r,i,-i
    CTq = nc.dram_tensor("CTq", [P, 3, KT, P], bf16, kind="ExternalInput")     # r,i,-i
    Af = nc.dram_tensor("Af", [P, 2, KT, N], f8, kind="ExternalInput")         # x32: r,i
    Cf = nc.dram_tensor("Cf", [P, 2, KT, N], f8, kind="ExternalInput")         # x16: r,i
    Yfr = nc.dram_tensor("Yfr", [P, 2, KT, N], bf16, kind="ExternalInput")     # r,i
    Yfb = nc.dram_tensor("Yfb", [P, 2, KT, N], f8, kind="ExternalInput")       # x16: r,i
    Wf = nc.dram_tensor("Wf", [P, 3, KT, N], bf16, kind="ExternalInput")       # r,i,r+i
    out = nc.dram_tensor("out", [2, P, N], f32, kind="ExternalOutput")

    with tile.TileContext(nc) as tc:
        with (
            tc.tile_pool(name="res", bufs=1) as res,          # residents + stationaries
            tc.tile_pool(name="stat", bufs=2) as statp,       # rotating transposed weights
            tc.tile_pool(name="work", bufs=2) as workp,       # rotating bf16 work tiles
            tc.tile_pool(name="mov", bufs=3) as movp,         # streamed moving tiles
            tc.tile_pool(name="tmp", bufs=3) as tmpp,         # combine temporaries
            tc.tile_pool(name="psum", bufs=6, space="PSUM") as ppool,
            tc.tile_pool(name="tpsum", bufs=2, space="PSUM") as tppool,
        ):
            identf = res.tile([P, P], f32, tag="identf")
            make_identity(nc, identf)
            identb = res.tile([P, P], bf16, tag="identb")
            nc.vector.tensor_copy(identb[:], identf[:])

            # PE warmup: keep the tensor engine busy (p-state ramp) while the
            # first DMAs land.  Results are discarded.
            for _ in range(5):
                wtp = tppool.tile([P, P], bf16, tag="tpb", name="wtp")
                nc.tensor.transpose(wtp[:], identb[:], identb[:])

            # sync(SP)-DGE: only the P1-critical small loads + the A stream
            ATw = res.tile([P, 3, KT, P], f8, tag="ATw")
            nc.sync.dma_start(ATw[:], ATq.ap())
            # scalar(Act)-DGE: C resident as ONE transfer (P1 runs all A2
            # matmuls before any V matmul, so Cres has ~13us to land and
            # chunk-granularity only added queue round-trips), then CTw.
            Cres = res.tile([P, 2, KT, N], f8, tag="Cres")
            nc.scalar.dma_start(Cres[:], Cf.ap())
            CTw = res.tile([P, 3, KT, P], bf16, tag="CTw")
            nc.scalar.dma_start(CTw[:], CTq.ap())

            afa = Af.ap()
            yra = Yfr.ap()
            yba = Yfb.ap()
            wfa = Wf.ap()

            def kara_combine(pk, cb):
                """pk = [P1, P2, P3] psums; cb(re_fn, im_fn) where the fns
                write re = P1-P2, im = P3-P1-P2 (<=1 psum operand per op)."""
                t1 = tmpp.tile([P, CW], f32, tag="kt", name="t1")
                nc.scalar.copy(t1[:], pk[0][:])
                t3 = tmpp.tile([P, CW], f32, tag="kt", name="t3")
                nc.scalar.copy(t3[:], pk[2][:])
                u = tmpp.tile([P, CW], f32, tag="kt", name="u")
                nc.vector.tensor_sub(u[:], t3[:], t1[:])
                cb(lambda dst: nc.vector.tensor_sub(dst, t1[:], pk[1][:]),
                   lambda dst: nc.vector.tensor_sub(dst, u[:], pk[1][:]))

            def mm4(pb, wt, mov0, mov1, st, sp, pm=None):
                """4-mult complex accumulate: wt planes (r, i, -i)."""
                nc.tensor.matmul(pb[0][:], wt[0], mov0, start=st, stop=False, perf_mode=pm)
                nc.tensor.matmul(pb[0][:], wt[2], mov1, start=False, stop=sp, perf_mode=pm)
                nc.tensor.matmul(pb[1][:], wt[0], mov1, start=st, stop=False, perf_mode=pm)
                nc.tensor.matmul(pb[1][:], wt[1], mov0, start=False, stop=sp, perf_mode=pm)

            # ---------------- P1: A2 = A@A + V = A@C (4m fp8) -------------
            A2b = workp.tile([P, 2, N], bf16, tag="wb", name="A2b")
            Vb = workp.tile([P, 2, N], bf16, tag="wb", name="Vb")
            for ci in range(NCH):
                cs = slice(CW * ci, CW * ci + CW)
                pa = [ppool.tile([P, CW], f32, tag="ps", name="pa") for _ in range(2)]
                pv = [ppool.tile([P, CW], f32, tag="ps", name="pv") for _ in range(2)]
                # all A2 matmuls (DMA-fed) before any V matmul (SBUF-fed):
                # an in-order PE queue must never park a V matmul that waits
                # on Cres in front of ready A2 work.  bufs=16 = total tiles,
                # so the whole 2MB A stream is issued ahead with no
                # buffer-rotation round-trips.
                for t in range(KT):
                    at = movp.tile([P, 2, CW], f8, tag="at", name="at", bufs=16)
                    nc.sync.dma_start(at[:], afa[:, :, t, cs])
                    wt = [ATw[:, j, t] for j in range(3)]
                    mm4(pa, wt, at[:, 0, :], at[:, 1, :], t == 0, t == KT - 1)
                for t in range(KT):
                    wt = [ATw[:, j, t] for j in range(3)]
                    mm4(pv, wt, Cres[:, 0, t, cs], Cres[:, 1, t, cs], t == 0, t == KT - 1)
                nc.scalar.mul(A2b[:, 0, cs], pa[0][:], 1.0 / (SA * SA))
                nc.vector.tensor_scalar_mul(A2b[:, 1, cs], pa[1][:], 1.0 / (SA * SA))
                nc.scalar.mul(Vb[:, 0, cs], pv[0][:], 1.0 / (SA * SC))
                nc.vector.tensor_scalar_mul(Vb[:, 1, cs], pv[1][:], 1.0 / (SA * SC))

            # W resident on the scalar HW-DGE, data-gated on P1's first
            # combine: a dummy tile shares Wres's buffer (same tag, bufs=1)
            # and is read by an op that depends on A2b, so the buffer-reuse
            # WAR dependency keeps the 6MB load from competing with the P1
            # streams for HBM.  (Engine-queue emission order alone is NOT a
            # schedule constraint - the tile list-scheduler reorders by
            # dependency readiness.)
            wdummy = res.tile([P, 3, KT, N], bf16, tag="Wres", name="wdummy")
            nc.gpsimd.memset(wdummy[0:1, 0, :, 0], 0.0)
            wgate = tmpp.tile([1, KT], f32, tag="wg", name="wgate", bufs=1)
            nc.vector.tensor_add(wgate[:], wdummy[0:1, 0, :, 0], A2b[0:1, 0, 0:KT])
            Wres = res.tile([P, 3, KT, N], bf16, tag="Wres", name="Wres")
            for t in range(KT):
                nc.scalar.dma_start(Wres[:, :, t, :], wfa[:, :, t, :])

            # A2T (fp8, x128, scalar-engine copies) and VT (f32r, vector-
            # engine copies) built in one interleaved loop so PE transposes
            # never wait on a single drain engine.
            A2T = res.tile([P, 3, KT, P], f8, tag="A2T")
            VT = res.tile([P, 3, KT, P], bf16, tag="VT")
            for t in range(KT):
                blk = slice(P * t, P * t + P)
                tpr = tppool.tile([P, P], bf16, tag="tpb", name="tpr")
                nc.tensor.transpose(tpr[:], A2b[:, 0, blk], identb[:])
                nc.scalar.mul(A2T[:, 0, t], tpr[:], SA2)
                tpi = tppool.tile([P, P], bf16, tag="tpb", name="tpi")
                nc.tensor.transpose(tpi[:], A2b[:, 1, blk], identb[:])
                nc.vector.tensor_scalar_mul(A2T[:, 1, t], tpi[:], SA2)
                nc.scalar.mul(A2T[:, 2, t], tpi[:], -SA2)
                vpr = tppool.tile([P, P], bf16, tag="tpb", name="vpr")
                nc.tensor.transpose(vpr[:], Vb[:, 0, blk], identb[:])
                nc.vector.tensor_copy(VT[:, 0, t], vpr[:])
                vpi = tppool.tile([P, P], bf16, tag="tpb", name="vpi")
                nc.tensor.transpose(vpi[:], Vb[:, 1, blk], identb[:])
                nc.vector.tensor_copy(VT[:, 1, t], vpi[:])
                nc.vector.tensor_scalar_mul(VT[:, 2, t], vpi[:], -1.0)

            # ---------------- P2: R = C@Y + U0 = V@Y + V2 = A2@C (4m) -----
            R32 = res.tile([P, 2, N], f32, tag="R32")
            U032 = res.tile([P, 2, N], f32, tag="U032")
            V2b = workp.tile([P, 2, N], bf16, tag="wb", name="V2b")
            for ci in range(NCH):
                cs = slice(CW * ci, CW * ci + CW)
                pr = [ppool.tile([P, CW], f32, tag="ps", name="pr") for _ in range(2)]
                pu = [ppool.tile([P, CW], f32, tag="ps", name="pu") for _ in range(2)]
                pv2 = [ppool.tile([P, CW], f32, tag="ps", name="pv2") for _ in range(2)]
                for t in range(KT):
                    yt = movp.tile([P, 2, CW], bf16, tag="yt", name="yt", bufs=6)
                    nc.sync.dma_start(yt[:], yra[:, :, t, cs])
                    st = t == 0
                    sp = t == KT - 1
                    mm4(pr, [CTw[:, j, t] for j in range(3)], yt[:, 0, :], yt[:, 1, :], st, sp)
                    mm4(pu, [VT[:, j, t] for j in range(3)], yt[:, 0, :], yt[:, 1, :], st, sp)
                    if t % 2 == 0:
                        # V2 in fp8 DoubleRow: one matmul covers the k-tile
                        # pair [t, t+1]; stationary/moving pair views are
                        # plain slices of the existing [.., KT, ..] layouts.
                        tp = slice(t, t + 2)
                        mm4(pv2, [A2T[:, j, tp, :] for j in range(3)],
                            Cres[:, 0, tp, cs], Cres[:, 1, tp, cs],
                            st, t == KT - 2, pm=DR)
                # drain the 6 banks on two engines in parallel
                nc.scalar.copy(R32[:, 0, cs], pr[0][:])
                nc.vector.tensor_copy(R32[:, 1, cs], pr[1][:])
                nc.scalar.copy(U032[:, 0, cs], pu[0][:])
                nc.vector.tensor_copy(U032[:, 1, cs], pu[1][:])
                nc.scalar.mul(V2b[:, 0, cs], pv2[0][:], 1.0 / (SA2 * SC))
                nc.vector.tensor_scalar_mul(V2b[:, 1, cs], pv2[1][:], 1.0 / (SA2 * SC))

            # V2T: fp8 x64 planes (r, i, -i) for the fp8 U2 pass
            V2T = statp.tile([P, 3, KT, P], f8, tag="v2t", name="V2T", bufs=1)
            for t in range(KT):
                blk = slice(P * t, P * t + P)
                tpr = tppool.tile([P, P], bf16, tag="tpb", name="w2r")
                nc.tensor.transpose(tpr[:], V2b[:, 0, blk], identb[:])
                nc.scalar.mul(V2T[:, 0, t], tpr[:], SV2)
                tpi = tppool.tile([P, P], bf16, tag="tpb", name="w2i")
                nc.tensor.transpose(tpi[:], V2b[:, 1, blk], identb[:])
                nc.vector.tensor_scalar_mul(V2T[:, 1, t], tpi[:], SV2)
                nc.scalar.mul(V2T[:, 2, t], tpi[:], -SV2)

            # ---------------- P3: U2 = V2@Y (4m fp8, DoubleRow) -----------
            # ybt pair-tiles get one buffer each (bufs=8 = total loads), so
            # the whole 2MB fp8 Y stream prefetches during P2 and P3 runs
            # with zero live DMA.
            U2b = workp.tile([P, 2, N], bf16, tag="wb", name="U2b")
            for ci in range(NCH):
                cs = slice(CW * ci, CW * ci + CW)
                pk = [ppool.tile([P, CW], f32, tag="ps", name="pk") for _ in range(2)]
                for tp2 in range(KT // 2):
                    tp = slice(2 * tp2, 2 * tp2 + 2)
                    ybt = movp.tile([P, 2, 2, CW], f8, tag="ybt", name="ybt", bufs=8)
                    nc.sync.dma_start(ybt[:], yba[:, :, tp, cs])
                    st = tp2 == 0
                    sp = tp2 == KT // 2 - 1
                    mm4(pk, [V2T[:, j, tp, :] for j in range(3)],
                        ybt[:, 0, :, :], ybt[:, 1, :, :], st, sp, pm=DR)
                nc.scalar.mul(U2b[:, 0, cs], pk[0][:], 1.0 / (SV2 * SY))
                nc.vector.tensor_scalar_mul(U2b[:, 1, cs], pk[1][:], 1.0 / (SV2 * SY))

            def tr_kara_t(src_bf, wt, t):
                """One k-tile of bf16 [P,2,N] -> Karatsuba planes (r,i,r+i)."""
                blk = slice(P * t, P * t + P)
                tpr = tppool.tile([P, P], bf16, tag="tpb", name="kr")
                nc.tensor.transpose(tpr[:], src_bf[:, 0, blk], identb[:])
                nc.scalar.copy(wt[:, 0, t], tpr[:])
                tpi = tppool.tile([P, P], bf16, tag="tpb", name="ki")
                nc.tensor.transpose(tpi[:], src_bf[:, 1, blk], identb[:])
                nc.vector.tensor_copy(wt[:, 1, t], tpi[:])
                nc.vector.tensor_add(wt[:, 2, t], wt[:, 0, t], tpi[:])

            U2T = statp.tile([P, 3, KT, P], bf16, tag="wt", name="U2T")
            for t in range(KT):
                tr_kara_t(U2b, U2T, t)

            # ---------------- P4: T2 = U2@W (kara); M = U0 + T2 -----------
            Mb = workp.tile([P, 2, N], bf16, tag="wb", name="Mb")
            for ci in range(NCH):
                cs = slice(CW * ci, CW * ci + CW)
                pk = [ppool.tile([P, CW], f32, tag="ps", name="pt") for _ in range(3)]
                # per-bank t-loops: bank b's accumulation finishes before
                # bank b+1's, so the combine's psum reads overlap the
                # remaining banks' matmuls instead of trailing all of them.
                for b in range(3):
                    for t in range(KT):
                        nc.tensor.matmul(pk[b][:], U2T[:, b, t], Wres[:, b, t, cs],
                                         start=t == 0, stop=t == KT - 1)

                def cbm(re, im, cs=cs):
                    rr = tmpp.tile([P, CW], f32, tag="kt", name="rr")
                    re(rr[:])
                    nc.vector.tensor_add(Mb[:, 0, cs], rr[:], U032[:, 0, cs])
                    ii = tmpp.tile([P, CW], f32, tag="kt", name="ii")
                    im(ii[:])
                    nc.vector.tensor_add(Mb[:, 1, cs], ii[:], U032[:, 1, cs])

                kara_combine(pk, cbm)

            MT = statp.tile([P, 3, KT, P], bf16, tag="wt", name="MT")
            for t in range(KT):
                tr_kara_t(Mb, MT, t)

            # ---------------- P5: S = R + M@W; write out ------------------
            oa = out.ap()
            for ci in range(NCH):
                cs = slice(CW * ci, CW * ci + CW)
                pk = [ppool.tile([P, CW], f32, tag="ps", name="pf") for _ in range(3)]
                for b in range(3):
                    for t in range(KT):
                        nc.tensor.matmul(pk[b][:], MT[:, b, t], Wres[:, b, t, cs],
                                         start=t == 0, stop=t == KT - 1)

                def cbf(re, im, cs=cs):
                    for j, part in ((0, re), (1, im)):
                        pp = tmpp.tile([P, CW], f32, tag="kt", name="pp")
                        part(pp[:])
                        og = tmpp.tile([P, CW], f32, tag="og", name="og", bufs=2)
                        nc.vector.tensor_add(og[:], pp[:], R32[:, j, cs])
                        nc.sync.dma_start(oa[j, :, cs], og[:])

                kara_combine(pk, cbf)

    nc.compile()
    return nc


def _prep_inputs(A, W, C, Y):
    import ml_dtypes
    bf = ml_dtypes.bfloat16
    f8 = ml_dtypes.float8_e4m3fn

    def full_layout(planes, dt):
        pl = np.stack(planes)  # [p, 1024, 1024]
        return np.ascontiguousarray(
            pl.reshape(len(planes), KT, P, N).transpose(2, 0, 1, 3).astype(dt))

    def shard_layout(M, c, planes_fn, dt):
        XT = M[P * c:P * c + P, :].T
        r = XT.real.astype(np.float32)
        i = XT.imag.astype(np.float32)
        pl = np.stack(planes_fn(r, i))  # [p, 1024, 128]
        npl = pl.shape[0]
        return np.ascontiguousarray(
            pl.reshape(npl, KT, P, P).transpose(2, 0, 1, 3).astype(dt))

    def re_im(M):
        return M.real.astype(np.float32), M.imag.astype(np.float32)

    Ar, Ai = re_im(A)
    Cr, Ci = re_im(C)
    Yr, Yi = re_im(Y)
    Wr, Wi = re_im(W)

    Af = full_layout([SA * Ar, SA * Ai], f8)
    Cfull = full_layout([SC * Cr, SC * Ci], f8)
    Yfr = full_layout([Yr, Yi], bf)
    Yfb = full_layout([SY * Yr, SY * Yi], f8)
    Wfull = full_layout([Wr, Wi, Wr + Wi], bf)

    in_maps = []
    for c in range(NC):
        in_maps.append({
            "ATq": shard_layout(A, c, lambda r, i: [SA * r, SA * i, -SA * i], f8),
            "CTq": shard_layout(C, c, lambda r, i: [r, i, -i], bf),
            "Af": Af, "Cf": Cfull, "Yfr": Yfr, "Yfb": Yfb, "Wf": Wfull,
        })
    return in_maps


def kernel(A, W, C, Y, _trace=False):
    from concourse import bass_utils

    if "nc" not in _compiled:
        _compiled["nc"] = _build()
    nc = _compiled["nc"]

    in_maps = _prep_inputs(A, W, C, Y)
    res = bass_utils.run_bass_kernel_spmd(
        nc, in_maps, core_ids=list(range(NC)), trace=_trace
    )
    _compiled["last_result"] = res

    full = np.empty((N, N), dtype=np.complex128)
    for c in range(NC):
        o = res.results[c]["out"]
        full[P * c:P * c + P, :] = o[0].astype(np.float64) + 1j * o[1].astype(np.float64)
    return full


# revision 50
# speedup vs baseline: 1.1494x; 1.0324x over previous
"""Stein solver  Lambda - A @ Lambda @ W = C @ Y  on 8 trn2 NeuronCores.

Math: Lambda = sum_k A^k R W^k with R = C@Y; per-step Frobenius contraction
of the series terms is ~0.08, so a 3-term truncation has exact error 5.1e-4
(gate is 2e-2).  Computed as

    S = R + (U0 + T2) @ W,   U0 = (A@C)@Y = A R,   T2 = ((A@A@C)@Y)@W = A^2 R W

which needs NO inter-core collectives: every GEMM is either full x full or
own-rows x full.

Distribution: row-sharded over 8 cores, core c owns rows [128c, 128c+128).
Five passes (496 matmuls + 80 transposes per core):
  P1: A2 = A@A + V = A@C, both 4-mult fp8(e4m3), sharing one fp8 A stream
      and the fp8 C resident; one shared 3-plane stationary (r, i, -i).
  P2: R = C@Y (bf16) + U0 = V@Y (bf16) + V2 = A2@C (fp8 DoubleRow),
      all 4-mult, sharing ONE streamed bf16-Y pass (6 of 8 PSUM banks).
  P3: U2 = V2@Y (4-mult fp8 DoubleRow, fully prefetched fp8 Y).
  P4: T2 = U2@W (Karatsuba bf16, W resident); combine writes M = U0 + T2.
  P5: S = R + M@W (Karatsuba bf16); combine adds R and DMAs the output.

fp8 keeps every pass under ~200 GB/s of live DMA (the bf16 version was
HBM-starved in P1/P3); operands carry host-folded power-of-2 scales
(A x32, C x16, Y x16, A2T x128, V2T x64) that are divided back out in the
PSUM-drain copies (scalar.mul is a free scaled copy).
fp8 terms enter the answer at <= 8e-2 relative scale; measured end-to-end
error of this exact scheme (CPU simulation): 3.9e-3 vs the 2e-2 gate.

DMA engine split: sync(SP)-DGE carries the latency-critical streams (A
tiles, Y tiles, output), scalar(Act)-DGE the C resident + CTw, and
gpsimd SWDGE the 6MB W resident, gated on P1's first combine so it cannot
steal HBM bandwidth from the P1 streams.  PSUM-sourced vector ops always
have at most one PSUM operand; drains alternate scalar/vector engines.
"""

import numpy as np

P = 128
N = 1024
KT = N // P          # 8 k-tiles
NC = 8               # cores
NCH = 2              # 512-wide n-chunks per 1024-col output row block
CW = N // NCH        # 512

SA = 32.0            # fp8 scale on A planes
SC = 16.0            # fp8 scale on C planes
SY = 16.0            # fp8 scale on Y planes
SA2 = 128.0          # fp8 scale on A2T planes
SV2 = 64.0           # fp8 scale on V2T planes

_compiled = {}


def _build():
    import concourse.mybir as mybir
    import concourse.tile as tile
    from concourse import bacc
    from concourse.masks import make_identity

    f32 = mybir.dt.float32
    f32r = mybir.dt.float32r
    bf16 = mybir.dt.bfloat16
    f8 = mybir.dt.float8e4
    DR = mybir.MatmulPerfMode.DoubleRow

    nc = bacc.Bacc("TRN2", target_bir_lowering=False, debug=False, num_devices=NC)

    # ---- I/O ----
    # full moving matrices laid out [partition, plane, ktile, col]:
    #   X[kt*128+p, c] at [p, j, kt, c]
    # sharded stationary [partition, plane, ktile, m]: (X[own,:].T) blocks
    ATq = nc.dram_tensor("ATq", [P, 3, KT, P], f8, kind="ExternalInput")       # x32: r,i,-i
    CTq = nc.dram_tensor("CTq", [P, 3, KT, P], bf16, kind="ExternalInput")     # r,i,-i
    Af = nc.dram_tensor("Af", [P, 2, KT, N], f8, kind="ExternalInput")         # x32: r,i
    Cf = nc.dram_tensor("Cf", [P, 2, KT, N], f8, kind="ExternalInput")         # x16: r,i
    Yfr = nc.dram_tensor("Yfr", [P, 2, KT, N], bf16, kind="ExternalInput")     # r,i
    Yfb = nc.dram_tensor("Yfb", [P, 2, KT, N], f8, kind="ExternalInput")       # x16: r,i
    Wf = nc.dram_tensor("Wf", [P, 3, KT, N], bf16, kind="ExternalInput")       # r,i,r+i
    out = nc.dram_tensor("out", [2, P, N], f32, kind="ExternalOutput")

    with tile.TileContext(nc) as tc:
        with (
            tc.tile_pool(name="res", bufs=1) as res,          # residents + stationaries
            tc.tile_pool(name="stat", bufs=2) as statp,       # rotating transposed weights
            tc.tile_pool(name="work", bufs=2) as workp,       # rotating bf16 work tiles
            tc.tile_pool(name="mov", bufs=3) as movp,         # streamed moving tiles
            tc.tile_pool(name="tmp", bufs=3) as tmpp,         # combine temporaries
            tc.tile_pool(name="psum", bufs=6, space="PSUM") as ppool,
            tc.tile_pool(name="tpsum", bufs=2, space="PSUM") as tppool,
        ):
            identf = res.tile([P, P], f32, tag="identf")
            make_identity(nc, identf)
            identb = res.tile([P, P], bf16, tag="identb")
            nc.vector.tensor_copy(identb[:], identf[:])

            # PE warmup: keep the tensor engine busy (p-state ramp) while the
            # first DMAs land.  Results are discarded.
            for _ in range(5):
                wtp = tppool.tile([P, P], bf16, tag="tpb", name="wtp")
                nc.tensor.transpose(wtp[:], identb[:], identb[:])

            # sync(SP)-DGE: only the P1-critical small loads + the A stream
            ATw = res.tile([P, 3, KT, P], f8, tag="ATw")
            nc.sync.dma_start(ATw[:], ATq.ap())
            # scalar(Act)-DGE: C resident as ONE transfer (P1 runs all A2
            # matmuls before any V matmul, so Cres has ~13us to land and
            # chunk-granularity only added queue round-trips), then CTw.
            Cres = res.tile([P, 2, KT, N], f8, tag="Cres")
            nc.scalar.dma_start(Cres[:], Cf.ap())
            CTw = res.tile([P, 3, KT, P], bf16, tag="CTw")
            nc.scalar.dma_start(CTw[:], CTq.ap())

            afa = Af.ap()
            yra = Yfr.ap()
            yba = Yfb.ap()
            wfa = Wf.ap()

            def kara_combine(pk, cb):
                """pk = [P1, P2, P3] psums; cb(re_fn, im_fn) where the fns
                write re = P1-P2, im = P3-P1-P2 (<=1 psum operand per op)."""
                t1 = tmpp.tile([P, CW], f32, tag="kt", name="t1")
                nc.scalar.copy(t1[:], pk[0][:])
                t3 = tmpp.tile([P, CW], f32, tag="kt", name="t3")
                nc.scalar.copy(t3[:], pk[2][:])
                u = tmpp.tile([P, CW], f32, tag="kt", name="u")
                nc.vector.tensor_sub(u[:], t3[:], t1[:])
                cb(lambda dst: nc.vector.tensor_sub(dst, t1[:], pk[1][:]),
                   lambda dst: nc.vector.tensor_sub(dst, u[:], pk[1][:]))

            def mm4(pb, wt, mov0, mov1, st, sp, pm=None):
                """4-mult complex accumulate: wt planes (r, i, -i)."""
                nc.tensor.matmul(pb[0][:], wt[0], mov0, start=st, stop=False, perf_mode=pm)
                nc.tensor.matmul(pb[0][:], wt[2], mov1, start=False, stop=sp, perf_mode=pm)
                nc.tensor.matmul(pb[1][:], wt[0], mov1, start=st, stop=False, perf_mode=pm)
                nc.tensor.matmul(pb[1][:], wt[1], mov0, start=False, stop=sp, perf_mode=pm)

            # ---------------- P1: A2 = A@A + V = A@C (4m fp8) -------------
            A2b = workp.tile([P, 2, N], bf16, tag="wb", name="A2b")
            Vb = workp.tile([P, 2, N], bf16, tag="wb", name="Vb")
            for ci in range(NCH):
                cs = slice(CW * ci, CW * ci + CW)
                pa = [ppool.tile([P, CW], f32, tag="ps", name="pa") for _ in range(2)]
                pv = [ppool.tile([P, CW], f32, tag="ps", name="pv") for _ in range(2)]
                # Both GEMMs in fp8 DoubleRow (one matmul per k-tile pair).
                # All A2 matmuls (DMA-fed) run before any V matmul
                # (SBUF-fed): an in-order PE queue must never park a V
                # matmul that waits on Cres in front of ready A2 work.
                # bufs=8 = total pair tiles, so the whole 2MB A stream is
                # issued ahead with no buffer-rotation round-trips.
                for tp2 in range(KT // 2):
                    tp = slice(2 * tp2, 2 * tp2 + 2)
                    at = movp.tile([P, 2, 2, CW], f8, tag="at", name="at", bufs=8)
                    nc.sync.dma_start(at[:], afa[:, :, tp, cs])
                    wt = [ATw[:, j, tp, :] for j in range(3)]
                    mm4(pa, wt, at[:, 0, :, :], at[:, 1, :, :],
                        tp2 == 0, tp2 == KT // 2 - 1, pm=DR)
                for tp2 in range(KT // 2):
                    tp = slice(2 * tp2, 2 * tp2 + 2)
                    wt = [ATw[:, j, tp, :] for j in range(3)]
                    mm4(pv, wt, Cres[:, 0, tp, cs], Cres[:, 1, tp, cs],
                        tp2 == 0, tp2 == KT // 2 - 1, pm=DR)
                nc.scalar.mul(A2b[:, 0, cs], pa[0][:], 1.0 / (SA * SA))
                nc.vector.tensor_scalar_mul(A2b[:, 1, cs], pa[1][:], 1.0 / (SA * SA))
                nc.scalar.mul(Vb[:, 0, cs], pv[0][:], 1.0 / (SA * SC))
                nc.vector.tensor_scalar_mul(Vb[:, 1, cs], pv[1][:], 1.0 / (SA * SC))

            # W resident on the scalar HW-DGE, data-gated on P1's first
            # combine: a dummy tile shares Wres's buffer (same tag, bufs=1)
            # and is read by an op that depends on A2b, so the buffer-reuse
            # WAR dependency keeps the 6MB load from competing with the P1
            # streams for HBM.  (Engine-queue emission order alone is NOT a
            # schedule constraint - the tile list-scheduler reorders by
            # dependency readiness.)
            wdummy = res.tile([P, 3, KT, N], bf16, tag="Wres", name="wdummy")
            nc.gpsimd.memset(wdummy[0:1, 0, :, 0], 0.0)
            wgate = tmpp.tile([1, KT], f32, tag="wg", name="wgate", bufs=1)
            nc.vector.tensor_add(wgate[:], wdummy[0:1, 0, :, 0], A2b[0:1, 0, 0:KT])
            Wres = res.tile([P, 3, KT, N], bf16, tag="Wres", name="Wres")
            for t in range(KT):
                nc.scalar.dma_start(Wres[:, :, t, :], wfa[:, :, t, :])

            # A2T (fp8, x128, scalar-engine copies) and VT (f32r, vector-
            # engine copies) built in one interleaved loop so PE transposes
            # never wait on a single drain engine.
            A2T = res.tile([P, 3, KT, P], f8, tag="A2T")
            VT = res.tile([P, 3, KT, P], bf16, tag="VT")
            for t in range(KT):
                blk = slice(P * t, P * t + P)
                tpr = tppool.tile([P, P], bf16, tag="tpb", name="tpr")
                nc.tensor.transpose(tpr[:], A2b[:, 0, blk], identb[:])
                nc.scalar.mul(A2T[:, 0, t], tpr[:], SA2)
                tpi = tppool.tile([P, P], bf16, tag="tpb", name="tpi")
                nc.tensor.transpose(tpi[:], A2b[:, 1, blk], identb[:])
                nc.vector.tensor_scalar_mul(A2T[:, 1, t], tpi[:], SA2)
                nc.scalar.mul(A2T[:, 2, t], tpi[:], -SA2)
                vpr = tppool.tile([P, P], bf16, tag="tpb", name="vpr")
                nc.tensor.transpose(vpr[:], Vb[:, 0, blk], identb[:])
                nc.vector.tensor_copy(VT[:, 0, t], vpr[:])
                vpi = tppool.tile([P, P], bf16, tag="tpb", name="vpi")
                nc.tensor.transpose(vpi[:], Vb[:, 1, blk], identb[:])
                nc.vector.tensor_copy(VT[:, 1, t], vpi[:])
                nc.vector.tensor_scalar_mul(VT[:, 2, t], vpi[:], -1.0)

            # ---------------- P2: R = C@Y + U0 = V@Y + V2 = A2@C (4m) -----
            R32 = res.tile([P, 2, N], f32, tag="R32")
            U032 = res.tile([P, 2, N], f32, tag="U032")
            V2b = workp.tile([P, 2, N], bf16, tag="wb", name="V2b")
            for ci in range(NCH):
                cs = slice(CW * ci, CW * ci + CW)
                pr = [ppool.tile([P, CW], f32, tag="ps", name="pr") for _ in range(2)]
                pu = [ppool.tile([P, CW], f32, tag="ps", name="pu") for _ in range(2)]
                pv2 = [ppool.tile([P, CW], f32, tag="ps", name="pv2") for _ in range(2)]
                for t in range(KT):
                    yt = movp.tile([P, 2, CW], bf16, tag="yt", name="yt", bufs=6)
                    nc.sync.dma_start(yt[:], yra[:, :, t, cs])
                    st = t == 0
                    sp = t == KT - 1
                    mm4(pr, [CTw[:, j, t] for j in range(3)], yt[:, 0, :], yt[:, 1, :], st, sp)
                    mm4(pu, [VT[:, j, t] for j in range(3)], yt[:, 0, :], yt[:, 1, :], st, sp)
                    if t % 2 == 0:
                        # V2 in fp8 DoubleRow: one matmul covers the k-tile
                        # pair [t, t+1]; stationary/moving pair views are
                        # plain slices of the existing [.., KT, ..] layouts.
                        tp = slice(t, t + 2)
                        mm4(pv2, [A2T[:, j, tp, :] for j in range(3)],
                            Cres[:, 0, tp, cs], Cres[:, 1, tp, cs],
                            st, t == KT - 2, pm=DR)
                # drain the 6 banks on two engines in parallel
                nc.scalar.copy(R32[:, 0, cs], pr[0][:])
                nc.vector.tensor_copy(R32[:, 1, cs], pr[1][:])
                nc.scalar.copy(U032[:, 0, cs], pu[0][:])
                nc.vector.tensor_copy(U032[:, 1, cs], pu[1][:])
                nc.scalar.mul(V2b[:, 0, cs], pv2[0][:], 1.0 / (SA2 * SC))
                nc.vector.tensor_scalar_mul(V2b[:, 1, cs], pv2[1][:], 1.0 / (SA2 * SC))

            # V2T: fp8 x64 planes (r, i, -i) for the fp8 U2 pass
            V2T = statp.tile([P, 3, KT, P], f8, tag="v2t", name="V2T", bufs=1)
            for t in range(KT):
                blk = slice(P * t, P * t + P)
                tpr = tppool.tile([P, P], bf16, tag="tpb", name="w2r")
                nc.tensor.transpose(tpr[:], V2b[:, 0, blk], identb[:])
                nc.scalar.mul(V2T[:, 0, t], tpr[:], SV2)
                tpi = tppool.tile([P, P], bf16, tag="tpb", name="w2i")
                nc.tensor.transpose(tpi[:], V2b[:, 1, blk], identb[:])
                nc.vector.tensor_scalar_mul(V2T[:, 1, t], tpi[:], SV2)
                nc.scalar.mul(V2T[:, 2, t], tpi[:], -SV2)

            # ---------------- P3: U2 = V2@Y (4m fp8, DoubleRow) -----------
            # ybt pair-tiles get one buffer each (bufs=8 = total loads), so
            # the whole 2MB fp8 Y stream prefetches during P2 and P3 runs
            # with zero live DMA.
            U2b = workp.tile([P, 2, N], bf16, tag="wb", name="U2b")
            for ci in range(NCH):
                cs = slice(CW * ci, CW * ci + CW)
                pk = [ppool.tile([P, CW], f32, tag="ps", name="pk") for _ in range(2)]
                for tp2 in range(KT // 2):
                    tp = slice(2 * tp2, 2 * tp2 + 2)
                    ybt = movp.tile([P, 2, 2, CW], f8, tag="ybt", name="ybt", bufs=8)
                    nc.sync.dma_start(ybt[:], yba[:, :, tp, cs])
                    st = tp2 == 0
                    sp = tp2 == KT // 2 - 1
                    mm4(pk, [V2T[:, j, tp, :] for j in range(3)],
                        ybt[:, 0, :, :], ybt[:, 1, :, :], st, sp, pm=DR)
                nc.scalar.mul(U2b[:, 0, cs], pk[0][:], 1.0 / (SV2 * SY))
                nc.vector.tensor_scalar_mul(U2b[:, 1, cs], pk[1][:], 1.0 / (SV2 * SY))

            def tr_kara_t(src_bf, wt, t):
                """One k-tile of bf16 [P,2,N] -> Karatsuba planes (r,i,r+i)."""
                blk = slice(P * t, P * t + P)
                tpr = tppool.tile([P, P], bf16, tag="tpb", name="kr")
                nc.tensor.transpose(tpr[:], src_bf[:, 0, blk], identb[:])
                nc.scalar.copy(wt[:, 0, t], tpr[:])
                tpi = tppool.tile([P, P], bf16, tag="tpb", name="ki")
                nc.tensor.transpose(tpi[:], src_bf[:, 1, blk], identb[:])
                nc.vector.tensor_copy(wt[:, 1, t], tpi[:])
                nc.vector.tensor_add(wt[:, 2, t], wt[:, 0, t], tpi[:])

            U2T = statp.tile([P, 3, KT, P], bf16, tag="wt", name="U2T")
            for t in range(KT):
                tr_kara_t(U2b, U2T, t)

            # ---------------- P4: T2 = U2@W (kara); M = U0 + T2 -----------
            Mb = workp.tile([P, 2, N], bf16, tag="wb", name="Mb")
            for ci in range(NCH):
                cs = slice(CW * ci, CW * ci + CW)
                pk = [ppool.tile([P, CW], f32, tag="ps", name="pt") for _ in range(3)]
                # per-bank t-loops: bank b's accumulation finishes before
                # bank b+1's, so the combine's psum reads overlap the
                # remaining banks' matmuls instead of trailing all of them.
                for b in range(3):
                    for t in range(KT):
                        nc.tensor.matmul(pk[b][:], U2T[:, b, t], Wres[:, b, t, cs],
                                         start=t == 0, stop=t == KT - 1)

                def cbm(re, im, cs=cs):
                    rr = tmpp.tile([P, CW], f32, tag="kt", name="rr")
                    re(rr[:])
                    nc.vector.tensor_add(Mb[:, 0, cs], rr[:], U032[:, 0, cs])
                    ii = tmpp.tile([P, CW], f32, tag="kt", name="ii")
                    im(ii[:])
                    nc.vector.tensor_add(Mb[:, 1, cs], ii[:], U032[:, 1, cs])

                kara_combine(pk, cbm)

            MT = statp.tile([P, 3, KT, P], bf16, tag="wt", name="MT")
            for t in range(KT):
                tr_kara_t(Mb, MT, t)

            # ---------------- P5: S = R + M@W; write out ------------------
            oa = out.ap()
            for ci in range(NCH):
                cs = slice(CW * ci, CW * ci + CW)
                pk = [ppool.tile([P, CW], f32, tag="ps", name="pf") for _ in range(3)]
                for b in range(3):
                    for t in range(KT):
                        nc.tensor.matmul(pk[b][:], MT[:, b, t], Wres[:, b, t, cs],
                                         start=t == 0, stop=t == KT - 1)

                def cbf(re, im, cs=cs):
                    for j, part in ((0, re), (1, im)):
                        pp = tmpp.tile([P, CW], f32, tag="kt", name="pp")
                        part(pp[:])
                        og = tmpp.tile([P, CW], f32, tag="og", name="og", bufs=2)
                        nc.vector.tensor_add(og[:], pp[:], R32[:, j, cs])
                        nc.sync.dma_start(oa[j, :, cs], og[:])

                kara_combine(pk, cbf)

    nc.compile()
    return nc


def _prep_inputs(A, W, C, Y):
    import ml_dtypes
    bf = ml_dtypes.bfloat16
    f8 = ml_dtypes.float8_e4m3fn

    def full_layout(planes, dt):
        pl = np.stack(planes)  # [p, 1024, 1024]
        return np.ascontiguousarray(
            pl.reshape(len(planes), KT, P, N).transpose(2, 0, 1, 3).astype(dt))

    def shard_layout(M, c, planes_fn, dt):
        XT = M[P * c:P * c + P, :].T
        r = XT.real.astype(np.float32)
        i = XT.imag.astype(np.float32)
        pl = np.stack(planes_fn(r, i))  # [p, 1024, 128]
        npl = pl.shape[0]
        return np.ascontiguousarray(
            pl.reshape(npl, KT, P, P).transpose(2, 0, 1, 3).astype(dt))

    def re_im(M):
        return M.real.astype(np.float32), M.imag.astype(np.float32)

    Ar, Ai = re_im(A)
    Cr, Ci = re_im(C)
    Yr, Yi = re_im(Y)
    Wr, Wi = re_im(W)

    Af = full_layout([SA * Ar, SA * Ai], f8)
    Cfull = full_layout([SC * Cr, SC * Ci], f8)
    Yfr = full_layout([Yr, Yi], bf)
    Yfb = full_layout([SY * Yr, SY * Yi], f8)
    Wfull = full_layout([Wr, Wi, Wr + Wi], bf)

    in_maps = []
    for c in range(NC):
        in_maps.append({
            "ATq": shard_layout(A, c, lambda r, i: [SA * r, SA * i, -SA * i], f8),
            "CTq": shard_layout(C, c, lambda r, i: [r, i, -i], bf),
            "Af": Af, "Cf": Cfull, "Yfr": Yfr, "Yfb": Yfb, "Wf": Wfull,
        })
    return in_maps


def kernel(A, W, C, Y, _trace=False):
    from concourse import bass_utils

    if "nc" not in _compiled:
        _compiled["nc"] = _build()
    nc = _compiled["nc"]

    in_maps = _prep_inputs(A, W, C, Y)
    res = bass_utils.run_bass_kernel_spmd(
        nc, in_maps, core_ids=list(range(NC)), trace=_trace
    )
    _compiled["last_result"] = res

    full = np.empty((N, N), dtype=np.complex128)
    for c in range(NC):
        o = res.results[c]["out"]
        full[P * c:P * c + P, :] = o[0].astype(np.float64) + 1j * o[1].astype(np.float64)
    return full
